# revision 4
# baseline (speedup 1.0000x reference)
"""Trainium2 Bass kernel for nn_CBModel_46926812676771 (scatter_memory).

Reference semantics: from two pose tensors [32, 18, 2] build four one-hot
heatmap stacks [2, 32, 18, 256, 256]:
  gen_poses[gi]  = heatmap of trunc'd sample-0 coords of pose{gi+1}, replicated over B
  step_poses[si] = heatmap of per-sample interpolated coords p1 + (si+1)*floor((p2-p1)/3)

Sharding: pure data parallel over B (4 samples per core, 8 cores).
Each core writes its 75.5 MB output shard: 288 one-hot [256,256] maps,
of which 180 are unique (gen maps are replicated over the 4 local samples).

Device strategy (memory-roofline bound):
  - per-map scatter target index t = 256*x + y (or -1 if out of bounds) is
    computed on-device from raw (x, y) coords with DVE ops (trunc via the
    +2^23 round trick plus floor/ceil correction).
  - unique maps live one-per-partition in two groups (128 + 52 slots); for
    each 8192-wide chunk j a single DVE tensor_scalar computes
    (iota - t[p]) == -j*8192, yielding the one-hot values directly.
  - HWDGE DMAs stream each chunk to HBM; gen chunks are written 4x (one per
    local batch index).
"""

import numpy as np

H = 256
W = 256
HWSZ = H * W  # 65536
B = 32
C = 18
NCORES = 8
BPC = B // NCORES  # 4
NSTACK = 2  # gen stacks / step stacks
F = 8192  # chunk free-dim size
NCHUNK = HWSZ // F
NROWS = NSTACK * BPC * C  # 144 rows per output tensor per core
MAGIC = 12582912.0  # 1.5 * 2^23: v + MAGIC lands in [2^23, 2^24) for |v| < 2^22

_PROG_CACHE = {}


def _build_program():
    import concourse.bacc as bacc
    import concourse.mybir as mybir
    import concourse.tile as tile

    f32 = mybir.dt.float32
    i32 = mybir.dt.int32
    Op = mybir.AluOpType

    nc = bacc.Bacc(
        "TRN2",
        target_bir_lowering=False,
        debug=False,
        enable_asserts=False,
        num_devices=NCORES,
    )
    coords_d = nc.dram_tensor("coords", [128, 4], f32, kind="ExternalInput")
    gen_d = nc.dram_tensor("gen_out", [NROWS, HWSZ], f32, kind="ExternalOutput")
    step_d = nc.dram_tensor("step_out", [NROWS, HWSZ], f32, kind="ExternalOutput")

    gen_ap = gen_d.ap()
    step_ap = step_d.ap()
    coords_ap = coords_d.ap()

    with tile.TileContext(nc) as tc:
        with (
            tc.tile_pool(name="const", bufs=1) as const,
            tc.tile_pool(name="outp", bufs=4) as outp,
        ):
            coords_sb = const.tile([128, 4], f32)
            nc.sync.dma_start(coords_sb[:], coords_ap[:, :])

            # iota 0..F-1 along the free dim, identical on every partition
            iota_i = outp.tile([128, F], i32, tag="ot")
            nc.gpsimd.iota(iota_i[:], pattern=[[1, F]], base=0, channel_multiplier=0)
            iota_f = const.tile([128, F], f32)
            nc.vector.tensor_copy(iota_f[:], iota_i[:])

            # scratch columns, 2 wide each (one per slot group)
            sc = const.tile([128, 48], f32)
            ncol = [0]

            def col():
                c0 = ncol[0]
                ncol[0] += 2
                return sc[:, c0 : c0 + 2]

            x_all = coords_sb[:, 0:2]
            y_all = coords_sb[:, 2:4]

            def emit_trunc(v):
                """Truncate toward zero (matches float->int c-cast semantics)."""
                rn = col()
                nc.vector.tensor_scalar(rn, v, MAGIC, None, Op.add)
                nc.vector.tensor_scalar(rn, rn, -MAGIC, None, Op.add)
                pos = col()
                nc.vector.tensor_scalar(pos, v, 0.0, None, Op.is_ge)
                fc = col()  # rn > v: round went up; floor needs -1
                nc.vector.tensor_tensor(fc, rn, v, Op.is_gt)
                cc = col()  # rn < v: round went down; ceil needs +1
                nc.vector.tensor_tensor(cc, rn, v, Op.is_lt)
                m1 = col()
                nc.vector.tensor_tensor(m1, pos, fc, Op.mult)
                m2 = col()
                nc.vector.tensor_tensor(m2, pos, cc, Op.mult)
                t = col()
                nc.vector.tensor_tensor(t, rn, m1, Op.subtract)
                nc.vector.tensor_tensor(t, t, cc, Op.add)
                nc.vector.tensor_tensor(t, t, m2, Op.subtract)
                return t

            xt = emit_trunc(x_all)
            yt = emit_trunc(y_all)
            xc = col()
            nc.vector.tensor_scalar(xc, xt, 0.0, 255.0, Op.max, Op.min)
            yc = col()
            nc.vector.tensor_scalar(yc, yt, 0.0, 255.0, Op.max, Op.min)
            vx = col()
            nc.vector.tensor_tensor(vx, xc, xt, Op.is_equal)
            vy = col()
            nc.vector.tensor_tensor(vy, yc, yt, Op.is_equal)
            valid = col()
            nc.vector.tensor_tensor(valid, vx, vy, Op.mult)
            # target = valid * (256*xc + yc + 1) - 1   (-1 never matches iota)
            t0 = col()
            nc.vector.tensor_scalar(t0, xc, 256.0, 1.0, Op.mult, Op.add)
            t1 = col()
            nc.vector.tensor_tensor(t1, t0, yc, Op.add)
            t2 = col()
            nc.vector.tensor_tensor(t2, t1, valid, Op.mult)
            target = col()
            nc.vector.tensor_scalar(target, t2, -1.0, None, Op.add)

            gen3 = gen_ap.rearrange("(s r) f -> s r f", s=NSTACK)

            for j in range(NCHUNK):
                lo = j * F
                hi = lo + F
                for g in range(2):
                    ot = outp.tile([128, F], f32, tag="ot")
                    # one-hot: (iota - target[p]) == -j*F
                    nc.vector.tensor_scalar(
                        ot[:],
                        iota_f[:],
                        target[:, g : g + 1],
                        float(-lo),
                        Op.subtract,
                        Op.is_equal,
                    )
                    if g == 0:
                        # group0 partitions 0..127 -> step rows 0..127
                        nc.sync.dma_start(step_ap[0:128, lo:hi], ot[:])
                    else:
                        # group1 partitions 0..15 -> step rows 128..143
                        nc.sync.dma_start(step_ap[128:NROWS, lo:hi], ot[0:16, :])
                        # partitions 16..51 -> gen rows, replicated over b
                        for s in range(NSTACK):
                            src = ot[16 + s * C : 16 + (s + 1) * C, :]
                            for b in range(BPC):
                                r0 = s * BPC * C + b * C
                                nc.sync.dma_start(
                                    gen_ap[r0 : r0 + C, lo:hi], src
                                )

    nc.compile()
    return nc


def _get_program():
    if "nc" not in _PROG_CACHE:
        _PROG_CACHE["nc"] = _build_program()
    return _PROG_CACHE["nc"]


def _pack_core_inputs(pose1_cor, pose2_cor):
    """Per-core [128, 4] float32 slot coords: cols [x_g0, x_g1, y_g0, y_g1]."""
    p1 = np.asarray(pose1_cor, np.float32)
    p2 = np.asarray(pose2_cor, np.float32)
    step = np.floor_divide(p2 - p1, np.float32(3.0)).astype(np.float32)
    c1 = p1 + step
    c2 = c1 + step
    # gen maps use sample-0 coords, identical on every core
    gen_coords = np.stack([p1[0], p2[0]], 0).reshape(NSTACK * C, 2)  # [36, 2]
    in_maps = []
    for k in range(NCORES):
        sl = slice(k * BPC, (k + 1) * BPC)
        step_coords = np.stack([c1[sl], c2[sl]], 0).reshape(NROWS, 2)  # [144, 2]
        g0 = step_coords[0:128]
        g1 = np.full((128, 2), -1.0e9, np.float32)
        g1[0:16] = step_coords[128:NROWS]
        g1[16:52] = gen_coords
        coords = np.empty((128, 4), np.float32)
        coords[:, 0] = g0[:, 0]
        coords[:, 1] = g1[:, 0]
        coords[:, 2] = g0[:, 1]
        coords[:, 3] = g1[:, 1]
        in_maps.append({"coords": coords})
    return in_maps


def _assemble(results):
    gen = np.concatenate(
        [r["gen_out"].reshape(NSTACK, BPC, C, H, W) for r in results], axis=1
    )
    step = np.concatenate(
        [r["step_out"].reshape(NSTACK, BPC, C, H, W) for r in results], axis=1
    )
    return gen, step


def kernel(pose1_cor, pose2_cor):
    from concourse.bass_utils import run_bass_kernel_spmd

    nc = _get_program()
    in_maps = _pack_core_inputs(pose1_cor, pose2_cor)
    res = run_bass_kernel_spmd(nc, in_maps, core_ids=list(range(NCORES)))
    return _assemble(res.results)
